# revision 35
# baseline (speedup 1.0000x reference)
"""Causal multi-head self-attention (B=1, S=4096, D=1024, H=16) on 8 NeuronCores.

Sharding: tensor-parallel over heads - each core owns 2 heads (Wq/Wk/Wv column
slices, Wo row slice), computes a partial output projection, and the host sums
the 8 partials (bf16 partials, fp32 host accumulation).

Perf design (v2, ~370us -> target <220us):
  The TRN2 PE clock-gate (HAM) defaults to 4/8 duty (1.2 GHz) and only
  sustained gap-free matmul activity releases it to 8/8 (2.4 GHz). The v1
  kernel ran projections as a separate phase, leaving the attention phase
  ACT-bound with PE micro-gaps -> HAM oscillation -> ~1.4 GHz effective.
  v2 fuses the phases: projections / V-transposes / output projections are
  emitted as *filler units* interleaved between attention groups so the PE
  instruction stream stays back-to-back:
    - prologue projects x-blocks 0,1; block b+2 is projected during block b's
      attention groups; output projections are deferred into blocks 6,7
      (where attention alone leaves the PE ACT-bound).
    - softmax denominators: DVE reciprocal_approx_fast (5x faster than
      InstReciprocal), bf16 convert on the idle GPSIMD engine.
    - normalization: ACT copies PSUM->bf16, DVE multiplies by the PE-broadcast
      reciprocal; h1 rows reach outT via SBUF-SBUF DMA.
    - y written bf16 (halves output DMA traffic).
    - bulk DMA triggering on the gpsimd + sync queues.

Device-side layout (per core) as v1: qT/kT in [channel, seq] with even/odd
channels de-interleaved per 32-row quarter so RoPE is 3 DVE ops per block;
v PE-transposed into v_all ([65 = dk64+ones] per head per k-block) so PV row
sums ride along in the PV matmul; scores^T layout with exp (no max-subtract,
scores are O(+-8)) and a static 128x128 triangle mask on diagonal blocks.
"""

import os
import sys
from collections import deque

import numpy as np

for _p in ("/opt/trn_rl_repo", "/root/.axon_site/_ro/trn_rl_repo"):
    if os.path.isdir(_p) and _p not in sys.path:
        sys.path.insert(0, _p)

import ml_dtypes

import concourse.bass as bass
import concourse.mybir as mybir
import concourse.tile as tile
from concourse import bacc
from concourse.bass_utils import run_bass_kernel_spmd
from concourse.masks import make_identity


def _install_ntff_shim():
    """The agent image's antenv lacks axon_hooks; provide it so
    run_bass_kernel_spmd(trace=True) can capture NTFF profiles."""
    try:
        from antenv import axon_hooks  # noqa: F401
        return
    except ImportError:
        pass
    try:
        import types
        import antenv
        from trn_agent_boot.trn_boot import _ntff_profile_via_ctypes
        so = "/opt/axon/libaxon_pjrt.so"
        if not os.path.exists(so):
            return
        hook = _ntff_profile_via_ctypes(so)
        m = types.ModuleType("antenv.axon_hooks")
        m._hook = hook
        m.set_axon_ntff_profile_hook = lambda h: setattr(m, "_hook", h)
        m.get_axon_ntff_profile_hook = lambda: m._hook
        sys.modules["antenv.axon_hooks"] = m
        antenv.axon_hooks = m
    except Exception:
        pass


_install_ntff_shim()

F32 = mybir.dt.float32
BF16 = mybir.dt.bfloat16
NPBF16 = ml_dtypes.bfloat16

S = 4096
D = 1024
H = 16
DK = 64
N_CORES = 8
SQ = 512          # q-block width (PSUM bank limit for fp32 N)
SK = 128          # k-block width (partition dim of scores^T)
GK = 2            # k-blocks fused per exp group (2 PSUM banks)
NB512 = S // SQ   # 8
NB128 = S // SK   # 32

# outproj units kept in reserve until blocks >= OPROJ_FREE_B (late blocks are
# ACT-bound; the reserve is PE filler there)
OPROJ_RESERVE = 9999
OPROJ_FREE_B = 6


def _emit(tc, xT, wqkT, wvT, woT, cosT, sinT, tri, perm, y, dbg=None):
    nc = tc.nc
    mul = mybir.AluOpType.mult
    addop = mybir.AluOpType.add
    exp = mybir.ActivationFunctionType.Exp
    ctx_pools = []

    # ---------------- persistent SBUF ----------------
    const = tc.tile_pool(name="const", bufs=1)
    big = tc.tile_pool(name="big", bufs=1)
    cp = const.__enter__()
    bp = big.__enter__()
    ctx_pools += [const, big]

    wqk_sb = cp.tile([128, 8, 256], BF16, tag="wqk")      # [part, kchunk, 256]
    wv_sb = cp.tile([128, 8, 128], BF16, tag="wv")
    wo_sb = cp.tile([128, 1024], BF16, tag="wo")
    cos_sb = cp.tile([128, S], BF16, tag="cos")
    sin_sb = cp.tile([128, S], BF16, tag="sin")
    tri_sb = cp.tile([128, 128], BF16, tag="tri")
    perm_sb = cp.tile([128, 128], BF16, tag="perm")
    ident_f = cp.tile([128, 128], F32, tag="ident_f")
    ident = cp.tile([128, 128], BF16, tag="ident")
    ones_sb = cp.tile([65, 64], BF16, tag="ones")
    onec = cp.tile([128, 1], BF16, tag="onec")

    qT = bp.tile([128, S], BF16, tag="qT")
    kT = bp.tile([128, S], BF16, tag="kT")
    v_all = bp.tile([128, NB128 * 130], BF16, tag="v_all")
    outT = bp.tile([128, S], BF16, tag="outT")

    # ---------------- early DMA: weights + first x blocks ----------------
    wqk_src = wqkT.ap().rearrange("(c p) n -> p c n", p=128)
    wv_src = wvT.ap().rearrange("(c p) n -> p c n", p=128)
    nc.sync.dma_start(wqk_sb[:, 0:2, :], wqk_src[:, 0:2, :])
    nc.gpsimd.dma_start(wv_sb[:, 0:2, :], wv_src[:, 0:2, :])

    xp = tc.tile_pool(name="xp", bufs=32)
    xpp = xp.__enter__()
    ctx_pools.append(xp)
    xts = {}

    def load_x(j, eng_pair):
        ts = []
        for kc in range(8):
            xt = xpp.tile([128, SQ], BF16, tag="x", name=f"x{j}_{kc}")
            eng = eng_pair[kc % 2]
            eng.dma_start(
                xt[:], xT.ap()[128 * kc:128 * (kc + 1), SQ * j:SQ * (j + 1)])
            ts.append(xt)
        xts[j] = ts

    load_x(0, (nc.sync, nc.gpsimd))
    nc.sync.dma_start(wqk_sb[:, 2:8, :], wqk_src[:, 2:8, :])
    nc.gpsimd.dma_start(wv_sb[:, 2:8, :], wv_src[:, 2:8, :])
    load_x(1, (nc.sync, nc.gpsimd))
    nc.gpsimd.dma_start(cos_sb[:], cosT.ap())
    nc.gpsimd.dma_start(sin_sb[:], sinT.ap())
    load_x(2, (nc.sync, nc.gpsimd))
    nc.gpsimd.dma_start(wo_sb[:], woT.ap())
    nc.gpsimd.dma_start(tri_sb[:], tri.ap())
    nc.gpsimd.dma_start(perm_sb[:], perm.ap())

    make_identity(nc, ident_f[:])
    nc.vector.tensor_copy(ident[:], ident_f[:])
    nc.vector.memset(ones_sb[64:65, :], 1.0)
    nc.vector.memset(onec[:], 1.0)

    # ones columns for row-sums: v_all[:, 130j+64] = v_all[:, 130j+129] = 1.0
    v_blk = v_all[:].rearrange("p (j c) -> p j c", c=130)
    nc.vector.tensor_copy(v_blk[:, :, 64], onec[:].broadcast_to([128, NB128]))
    nc.vector.tensor_copy(v_blk[:, :, 129], onec[:].broadcast_to([128, NB128]))

    # ---------------- working pools ----------------
    pools = dict(
        vtst=tc.tile_pool(name="vtst", bufs=2),
        ropep=tc.tile_pool(name="ropep", bufs=2),
        ptp=tc.tile_pool(name="ptp", bufs=4),
        recp=tc.tile_pool(name="recp", bufs=2),
        nrmp=tc.tile_pool(name="nrmp", bufs=2),
        t64p=tc.tile_pool(name="t64p", bufs=2),
        ysbp=tc.tile_pool(name="ysb", bufs=4),
        scps=tc.tile_pool(name="scps", bufs=2, space="PSUM"),
        smps=tc.tile_pool(name="smps", bufs=2, space="PSUM"),
        fps=tc.tile_pool(name="fps", bufs=2, space="PSUM"),
    )
    p = {k: v.__enter__() for k, v in pools.items()}
    ctx_pools += list(pools.values())

    vt_stage = {}

    dbg_nr = None
    if dbg is not None:
        dbg_nr = cp.tile([128, 16 * SQ], BF16, tag="dbg_nr")
        nc.vector.memset(dbg_nr[:], 0.0)

    # ---------------- filler units ----------------
    def qkv_pass(j, which):
        def unit():
            ps = p["fps"].tile([128, SQ], F32, tag="fps", name=f"p{which}_{j}")
            for kc in range(8):
                if which == 0:
                    w = wqk_sb[:, kc, 0:128]
                elif which == 1:
                    w = wqk_sb[:, kc, 128:256]
                else:
                    w = wv_sb[:, kc, :]
                nc.tensor.matmul(ps[:], w, xts[j][kc][:],
                                 start=(kc == 0), stop=(kc == 7))
            sl = slice(SQ * j, SQ * (j + 1))
            if which == 0:
                nc.scalar.copy(qT[:, sl], ps[:])
            elif which == 1:
                nc.scalar.copy(kT[:, sl], ps[:])
            else:
                vt = p["vtst"].tile([128, SQ], BF16, tag="vt", name=f"vt{j}")
                nc.vector.tensor_copy(vt[:], ps[:])
                vt_stage[j] = vt
        return unit

    def transp_unit(j):
        def unit():
            tp = p["fps"].tile([128, SQ], BF16, tag="fps", name=f"tp{j}")
            vt = vt_stage.pop(j)
            for t4 in range(4):
                csl = slice(128 * t4, 128 * (t4 + 1))
                nc.tensor.transpose(tp[:, csl], vt[:, csl], ident[:])
                jj = 4 * j + t4
                dst = v_all[:, 130 * jj:130 * jj + 130].rearrange(
                    "p (t c) -> p t c", c=65)[:, :, 0:64]
                src = tp[:, csl].rearrange("p (t c) -> p t c", c=64)
                nc.vector.tensor_copy(dst, src)
        return unit

    def rope_unit(j, which):
        # the x1/x2 32-row stripe swap runs on the PE as a permutation
        # matmul (an SBUF->SBUF DMA here loses a race against the DVE
        # consumer: sparse NaNs in the first stripe)
        def unit():
            t = qT if which == 0 else kT
            sl = slice(SQ * j, SQ * (j + 1))
            sw = p["fps"].tile([128, SQ], F32, tag="fps", name=f"sw{which}_{j}")
            nc.tensor.matmul(sw[:], perm_sb[:], t[:, sl], start=True, stop=True)
            ta = p["ropep"].tile([128, SQ], BF16, tag="ta", name=f"ta{which}_{j}")
            tb = p["ropep"].tile([128, SQ], BF16, tag="tb", name=f"tb{which}_{j}")
            nc.vector.tensor_tensor(ta[:], t[:, sl], cos_sb[:, sl], op=mul)
            nc.vector.tensor_tensor(tb[:], sw[:], sin_sb[:, sl], op=mul)
            nc.vector.tensor_tensor(t[:, sl], ta[:], tb[:], op=addop)
        return unit

    def norm_unit(b, h, out_ps, rec):
        def unit():
            qsl = slice(SQ * b, SQ * (b + 1))
            bc = p["fps"].tile([64, SQ], F32, tag="fps", name=f"bc{b}_{h}")
            for hf in (slice(0, SQ // 2), slice(SQ // 2, SQ)):
                nc.tensor.matmul(bc[:, hf], ones_sb[64:65, :], rec[64:65, hf],
                                 start=True, stop=True)
            nrm = p["nrmp"].tile([64, SQ], BF16, tag="nrm", name=f"nr{b}_{h}")
            nc.scalar.copy(nrm[:], out_ps[0:64, :])
            if dbg_nr is not None:
                dsl = slice(SQ * (2 * b + h), SQ * (2 * b + h + 1))
                nc.vector.tensor_copy(dbg_nr[0:64, dsl], nrm[:])
                nc.vector.tensor_copy(dbg_nr[64:65, dsl], rec[64:65, :])
            t64 = p["t64p"].tile([64, SQ], BF16, tag="t64", name=f"t6{b}_{h}")
            nc.vector.tensor_tensor(t64[:], nrm[:], bc[:], op=mul)
            nc.sync.dma_start(outT[64 * h:64 * h + 64, qsl], t64[:])
        return unit

    def oproj_unit(b, m):
        def unit():
            msl = slice(SQ * b + 128 * m, SQ * b + 128 * (m + 1))
            for nh2 in range(2):
                nsl = slice(512 * nh2, 512 * (nh2 + 1))
                y_ps = p["fps"].tile([128, SQ], F32, tag="fps",
                                     name=f"y{b}_{m}_{nh2}")
                nc.tensor.matmul(y_ps[:], outT[:, msl], wo_sb[:, nsl],
                                 start=True, stop=True)
                y_sb = p["ysbp"].tile([128, SQ], BF16, tag="ysb",
                                      name=f"ys{b}_{m}_{nh2}")
                nc.vector.tensor_copy(y_sb[:], y_ps[:])
                (nc.sync if nh2 == 0 else nc.gpsimd).dma_start(
                    y.ap()[msl, nsl], y_sb[:])
        return unit

    # ---------------- prologue: project blocks 0 and 1 inline ----------------
    for j in (0, 1):
        for which in range(3):
            qkv_pass(j, which)()
        transp_unit(j)()
        rope_unit(j, 0)()
        rope_unit(j, 1)()

    # ---------------- fused attention + filler main loop ----------------
    fq = deque()       # high-priority filler (projections, norms)
    fq2 = deque()      # output-projection backlog

    def emit_units(b):
        n = 0
        while fq and n < 3:
            fq.popleft()()
            n += 1
        reserve = OPROJ_RESERVE if b < OPROJ_FREE_B else 0
        while fq2 and len(fq2) > reserve and n < 3:
            fq2.popleft()()
            n += 1

    for b in range(NB512):
        j = b + 2
        if j < NB512:
            fq.append(qkv_pass(j, 0))
            fq.append(qkv_pass(j, 1))
            fq.append(qkv_pass(j, 2))
            fq.append(transp_unit(j))
            fq.append(rope_unit(j, 0))
            fq.append(rope_unit(j, 1))
        if b + 3 < NB512:
            load_x(b + 3, (nc.sync, nc.gpsimd))

        nk = 4 * b + 4
        qsl = slice(SQ * b, SQ * (b + 1))
        for h in range(2):
            rh = slice(64 * h, 64 * h + 64)
            out_ps = p["smps"].tile([65, SQ], F32, tag="sm", name=f"o{b}_{h}")

            def emit_pv(pt, lo, g0):
                for j2 in range(GK):
                    k = g0 + j2
                    nc.tensor.matmul(out_ps[:, lo[j2]:SQ],
                                     v_all[:, 130 * k + 65 * h:
                                           130 * k + 65 * h + 65],
                                     pt[:, SQ * j2 + lo[j2]:SQ * (j2 + 1)],
                                     start=(k == 0), stop=(k == nk - 1))

            prev = None
            for g0 in range(0, nk, GK):
                sc = p["scps"].tile([128, GK * SQ], F32, tag="sc",
                                    name=f"sc{b}_{h}_{g0}")
                lo = {}
                for j2 in range(GK):
                    k = g0 + j2
                    lo[j2] = 128 * (k - 4 * b) if k >= 4 * b else 0
                    # write-lo: j2=1 always writes its full 512 columns so
                    # the single per-group activation reads a contiguous,
                    # fully-written [lo0, 1024) range (PV/tri still use the
                    # real lo; the extra columns are never streamed).
                    wlo = lo[j2] if j2 == 0 else 0
                    ksl = slice(SK * k, SK * (k + 1))
                    osl = slice(SQ * j2 + wlo, SQ * (j2 + 1))
                    nc.tensor.matmul(sc[:, osl], kT[rh, ksl],
                                     qT[rh, slice(SQ * b + wlo,
                                                  SQ * (b + 1))],
                                     start=True, stop=True)
                pt = p["ptp"].tile([128, GK * SQ], BF16, tag="pt",
                                   name=f"pt{b}_{h}_{g0}")
                nc.scalar.activation(pt[:, lo[0]:SQ * GK], sc[:, lo[0]:SQ * GK],
                                     exp, scale=0.125)
                for j2 in range(GK):
                    k = g0 + j2
                    if k >= 4 * b:
                        dsl = slice(SQ * j2 + lo[j2], SQ * j2 + lo[j2] + 128)
                        nc.vector.tensor_tensor(pt[:, dsl], pt[:, dsl],
                                                tri_sb[:], op=mul)
                emit_units(b)
                if prev is not None:
                    emit_pv(*prev)
                prev = (pt, dict(lo), g0)
            emit_pv(*prev)

            rec_f = p["recp"].tile([65, SQ], F32, tag="rec_f", name=f"rf{b}{h}")
            rec = p["recp"].tile([65, SQ], BF16, tag="rec", name=f"rc{b}{h}")
            nc.vector.reciprocal(rec_f[64:65, :], out_ps[64:65, :])
            nc.vector.tensor_copy(rec[64:65, :], rec_f[64:65, :])
            fq.appendleft(norm_unit(b, h, out_ps, rec))
        for m in range(4):
            fq2.append(oproj_unit(b, m))

    # ---------------- epilogue ----------------
    while fq:
        fq.popleft()()
    while fq2:
        fq2.popleft()()

    if dbg is not None:
        # qT/kT slots repurposed to carry the norm-unit dump (nrm rows 0:64,
        # rec row 64) for bh 0-7 and 8-15 respectively
        nc.sync.dma_start(dbg["qT"].ap(), dbg_nr[:, 0:S])
        nc.sync.dma_start(dbg["kT"].ap(), dbg_nr[:, S:2 * S])
        nc.sync.dma_start(dbg["v_all"].ap(), v_all[:])
        nc.sync.dma_start(dbg["outT"].ap(), outT[:])

    for pl in reversed(ctx_pools):
        pl.__exit__(None, None, None)


_CACHED = None


def _build():
    global _CACHED
    if _CACHED is not None:
        return _CACHED
    nc = bacc.Bacc("TRN2", target_bir_lowering=False, debug=False)
    xT = nc.dram_tensor("xT", [D, S], BF16, kind="ExternalInput")
    wqkT = nc.dram_tensor("wqkT", [D, 256], BF16, kind="ExternalInput")
    wvT = nc.dram_tensor("wvT", [D, 128], BF16, kind="ExternalInput")
    woT = nc.dram_tensor("woT", [128, D], BF16, kind="ExternalInput")
    cosT = nc.dram_tensor("cosT", [128, S], BF16, kind="ExternalInput")
    sinT = nc.dram_tensor("sinT", [128, S], BF16, kind="ExternalInput")
    tri = nc.dram_tensor("tri", [128, 128], BF16, kind="ExternalInput")
    perm = nc.dram_tensor("perm", [128, 128], BF16, kind="ExternalInput")
    y = nc.dram_tensor("y", [S, D], BF16, kind="ExternalOutput")
    dbg = None
    if os.environ.get("KERN_DEBUG"):
        dbg = {
            "qT": nc.dram_tensor("dbg_qT", [128, S], BF16, kind="ExternalOutput"),
            "kT": nc.dram_tensor("dbg_kT", [128, S], BF16, kind="ExternalOutput"),
            "v_all": nc.dram_tensor("dbg_v_all", [128, NB128 * 130], BF16,
                                    kind="ExternalOutput"),
            "outT": nc.dram_tensor("dbg_outT", [128, S], BF16,
                                   kind="ExternalOutput"),
        }
    with tile.TileContext(nc) as tc:
        _emit(tc, xT, wqkT, wvT, woT, cosT, sinT, tri, perm, y, dbg=dbg)
    nc.compile()
    _CACHED = nc
    return nc


def _host_prep(x, token_positions, Wq, Wk, Wv, Wo):
    x = np.asarray(x, dtype=np.float32).reshape(S, D)
    xT = np.ascontiguousarray(x.T).astype(NPBF16)

    pos = np.asarray(token_positions).reshape(S).astype(np.float32)
    inv = (np.float32(10000.0) **
           (-np.arange(0, DK // 2, dtype=np.float32) * np.float32(2.0 / DK)))
    ang = pos[None, :] * inv[:, None]          # [32, S]
    cosF = np.cos(ang).astype(np.float32)
    sinF = np.sin(ang).astype(np.float32)
    cosT = np.ascontiguousarray(np.tile(cosF, (4, 1))).astype(NPBF16)
    sinT = np.ascontiguousarray(np.tile(
        np.concatenate([-sinF, sinF], axis=0), (2, 1))).astype(NPBF16)  # signed

    ii = np.arange(128)[:, None]
    uu = np.arange(128)[None, :]
    tri = (uu >= ii).astype(NPBF16)             # [128, 128] triangle

    # stripe-swap permutation for RoPE: out[m] = in[m with 32-row block
    # pairs swapped]; used as matmul stationary (perm[k, m] = 1 iff k = swap(m))
    swap = (np.arange(128) // 32 ^ 1) * 32 + np.arange(128) % 32
    perm = np.zeros((128, 128), dtype=NPBF16)
    perm[swap, np.arange(128)] = 1.0

    Wq = np.asarray(Wq, dtype=np.float32)
    Wk = np.asarray(Wk, dtype=np.float32)
    Wv = np.asarray(Wv, dtype=np.float32)
    Wo = np.asarray(Wo, dtype=np.float32)

    in_maps = []
    for c in range(N_CORES):
        idx = []
        for hl in range(2):   # per head: 32 even channels then 32 odd channels
            idx += [64 * (2 * c + hl) + 2 * j for j in range(32)]
            idx += [64 * (2 * c + hl) + 2 * j + 1 for j in range(32)]
        wq_c = Wq[idx, :]                       # [128, 1024]
        wk_c = Wk[idx, :]
        wqkT = np.ascontiguousarray(
            np.concatenate([wq_c.T, wk_c.T], axis=1)).astype(NPBF16)  # [1024, 256]
        wvT = np.ascontiguousarray(
            Wv[128 * c:128 * (c + 1), :].T).astype(NPBF16)  # [1024, 128]
        woT = np.ascontiguousarray(
            Wo[:, 128 * c:128 * (c + 1)].T).astype(NPBF16)  # [128, 1024]
        in_maps.append({
            "xT": xT, "wqkT": wqkT, "wvT": wvT, "woT": woT,
            "cosT": cosT, "sinT": sinT, "tri": tri, "perm": perm,
        })
    return in_maps


def run(x, token_positions, Wq, Wk, Wv, Wo, trace=False):
    nc = _build()
    in_maps = _host_prep(x, token_positions, Wq, Wk, Wv, Wo)
    res = run_bass_kernel_spmd(nc, in_maps, core_ids=list(range(N_CORES)),
                               trace=trace)
    y = np.zeros((S, D), dtype=np.float32)
    for c in range(N_CORES):
        y += np.asarray(res.results[c]["y"], dtype=np.float32)
    return y.reshape(1, S, D), res


def kernel(x, token_positions, Wq, Wk, Wv, Wo):
    y, _ = run(x, token_positions, Wq, Wk, Wv, Wo)
    return y


# revision 36
# speedup vs baseline: 1.0290x; 1.0290x over previous
"""Causal multi-head self-attention (B=1, S=4096, D=1024, H=16) on 8 NeuronCores.

Sharding: tensor-parallel over heads - each core owns 2 heads (Wq/Wk/Wv column
slices, Wo row slice), computes a partial output projection, and the host sums
the 8 partials (bf16 partials, fp32 host accumulation).

Perf design (v2, ~370us -> target <220us):
  The TRN2 PE clock-gate (HAM) defaults to 4/8 duty (1.2 GHz) and only
  sustained gap-free matmul activity releases it to 8/8 (2.4 GHz). The v1
  kernel ran projections as a separate phase, leaving the attention phase
  ACT-bound with PE micro-gaps -> HAM oscillation -> ~1.4 GHz effective.
  v2 fuses the phases: projections / V-transposes / output projections are
  emitted as *filler units* interleaved between attention groups so the PE
  instruction stream stays back-to-back:
    - prologue projects x-blocks 0,1; block b+2 is projected during block b's
      attention groups; output projections are deferred into blocks 6,7
      (where attention alone leaves the PE ACT-bound).
    - softmax denominators: DVE reciprocal_approx_fast (5x faster than
      InstReciprocal), bf16 convert on the idle GPSIMD engine.
    - normalization: ACT copies PSUM->bf16, DVE multiplies by the PE-broadcast
      reciprocal; h1 rows reach outT via SBUF-SBUF DMA.
    - y written bf16 (halves output DMA traffic).
    - bulk DMA triggering on the gpsimd + sync queues.

Device-side layout (per core) as v1: qT/kT in [channel, seq] with even/odd
channels de-interleaved per 32-row quarter so RoPE is 3 DVE ops per block;
v PE-transposed into v_all ([65 = dk64+ones] per head per k-block) so PV row
sums ride along in the PV matmul; scores^T layout with exp (no max-subtract,
scores are O(+-8)) and a static 128x128 triangle mask on diagonal blocks.
"""

import os
import sys
from collections import deque

import numpy as np

for _p in ("/opt/trn_rl_repo", "/root/.axon_site/_ro/trn_rl_repo"):
    if os.path.isdir(_p) and _p not in sys.path:
        sys.path.insert(0, _p)

import ml_dtypes

import concourse.bass as bass
import concourse.mybir as mybir
import concourse.tile as tile
from concourse import bacc
from concourse.bass_utils import run_bass_kernel_spmd
from concourse.masks import make_identity


def _install_ntff_shim():
    """The agent image's antenv lacks axon_hooks; provide it so
    run_bass_kernel_spmd(trace=True) can capture NTFF profiles."""
    try:
        from antenv import axon_hooks  # noqa: F401
        return
    except ImportError:
        pass
    try:
        import types
        import antenv
        from trn_agent_boot.trn_boot import _ntff_profile_via_ctypes
        so = "/opt/axon/libaxon_pjrt.so"
        if not os.path.exists(so):
            return
        hook = _ntff_profile_via_ctypes(so)
        m = types.ModuleType("antenv.axon_hooks")
        m._hook = hook
        m.set_axon_ntff_profile_hook = lambda h: setattr(m, "_hook", h)
        m.get_axon_ntff_profile_hook = lambda: m._hook
        sys.modules["antenv.axon_hooks"] = m
        antenv.axon_hooks = m
    except Exception:
        pass


_install_ntff_shim()

F32 = mybir.dt.float32
BF16 = mybir.dt.bfloat16
NPBF16 = ml_dtypes.bfloat16

S = 4096
D = 1024
H = 16
DK = 64
N_CORES = 8
SQ = 512          # q-block width (PSUM bank limit for fp32 N)
SK = 128          # k-block width (partition dim of scores^T)
GK = 2            # k-blocks fused per exp group (2 PSUM banks)
NB512 = S // SQ   # 8
NB128 = S // SK   # 32

# outproj units kept in reserve until blocks >= OPROJ_FREE_B (late blocks are
# ACT-bound; the reserve is PE filler there)
OPROJ_RESERVE = 12
OPROJ_FREE_B = 6


def _emit(tc, xT, wqkT, wvT, woT, cosT, sinT, tri, perm, y, dbg=None):
    nc = tc.nc
    mul = mybir.AluOpType.mult
    addop = mybir.AluOpType.add
    exp = mybir.ActivationFunctionType.Exp
    ctx_pools = []

    # ---------------- persistent SBUF ----------------
    const = tc.tile_pool(name="const", bufs=1)
    big = tc.tile_pool(name="big", bufs=1)
    cp = const.__enter__()
    bp = big.__enter__()
    ctx_pools += [const, big]

    wqk_sb = cp.tile([128, 8, 256], BF16, tag="wqk")      # [part, kchunk, 256]
    wv_sb = cp.tile([128, 8, 128], BF16, tag="wv")
    wo_sb = cp.tile([128, 1024], BF16, tag="wo")
    cos_sb = cp.tile([128, S], BF16, tag="cos")
    sin_sb = cp.tile([128, S], BF16, tag="sin")
    tri_sb = cp.tile([128, 128], BF16, tag="tri")
    perm_sb = cp.tile([128, 128], BF16, tag="perm")
    ident_f = cp.tile([128, 128], F32, tag="ident_f")
    ident = cp.tile([128, 128], BF16, tag="ident")
    ones_sb = cp.tile([65, 64], BF16, tag="ones")
    onec = cp.tile([128, 1], BF16, tag="onec")

    qT = bp.tile([128, S], BF16, tag="qT")
    kT = bp.tile([128, S], BF16, tag="kT")
    v_all = bp.tile([128, NB128 * 130], BF16, tag="v_all")
    outT = bp.tile([128, S], BF16, tag="outT")

    # ---------------- early DMA: weights + first x blocks ----------------
    wqk_src = wqkT.ap().rearrange("(c p) n -> p c n", p=128)
    wv_src = wvT.ap().rearrange("(c p) n -> p c n", p=128)
    nc.sync.dma_start(wqk_sb[:, 0:2, :], wqk_src[:, 0:2, :])
    nc.gpsimd.dma_start(wv_sb[:, 0:2, :], wv_src[:, 0:2, :])

    xp = tc.tile_pool(name="xp", bufs=32)
    xpp = xp.__enter__()
    ctx_pools.append(xp)
    xts = {}

    def load_x(j, eng_pair):
        ts = []
        for kc in range(8):
            xt = xpp.tile([128, SQ], BF16, tag="x", name=f"x{j}_{kc}")
            eng = eng_pair[kc % 2]
            eng.dma_start(
                xt[:], xT.ap()[128 * kc:128 * (kc + 1), SQ * j:SQ * (j + 1)])
            ts.append(xt)
        xts[j] = ts

    load_x(0, (nc.sync, nc.gpsimd))
    nc.sync.dma_start(wqk_sb[:, 2:8, :], wqk_src[:, 2:8, :])
    nc.gpsimd.dma_start(wv_sb[:, 2:8, :], wv_src[:, 2:8, :])
    load_x(1, (nc.sync, nc.gpsimd))
    nc.gpsimd.dma_start(cos_sb[:], cosT.ap())
    nc.gpsimd.dma_start(sin_sb[:], sinT.ap())
    load_x(2, (nc.sync, nc.gpsimd))
    nc.gpsimd.dma_start(wo_sb[:], woT.ap())
    nc.gpsimd.dma_start(tri_sb[:], tri.ap())
    nc.gpsimd.dma_start(perm_sb[:], perm.ap())

    make_identity(nc, ident_f[:])
    nc.vector.tensor_copy(ident[:], ident_f[:])
    nc.vector.memset(ones_sb[64:65, :], 1.0)
    nc.vector.memset(onec[:], 1.0)

    # ones columns for row-sums: v_all[:, 130j+64] = v_all[:, 130j+129] = 1.0
    v_blk = v_all[:].rearrange("p (j c) -> p j c", c=130)
    nc.vector.tensor_copy(v_blk[:, :, 64], onec[:].broadcast_to([128, NB128]))
    nc.vector.tensor_copy(v_blk[:, :, 129], onec[:].broadcast_to([128, NB128]))

    # ---------------- working pools ----------------
    pools = dict(
        vtst=tc.tile_pool(name="vtst", bufs=2),
        ropep=tc.tile_pool(name="ropep", bufs=2),
        ptp=tc.tile_pool(name="ptp", bufs=4),
        recp=tc.tile_pool(name="recp", bufs=2),
        nrmp=tc.tile_pool(name="nrmp", bufs=2),
        t64p=tc.tile_pool(name="t64p", bufs=2),
        ysbp=tc.tile_pool(name="ysb", bufs=4),
        scps=tc.tile_pool(name="scps", bufs=2, space="PSUM"),
        smps=tc.tile_pool(name="smps", bufs=2, space="PSUM"),
        fps=tc.tile_pool(name="fps", bufs=2, space="PSUM"),
    )
    p = {k: v.__enter__() for k, v in pools.items()}
    ctx_pools += list(pools.values())

    vt_stage = {}

    dbg_nr = None
    if dbg is not None:
        dbg_nr = cp.tile([128, 16 * SQ], BF16, tag="dbg_nr")
        nc.vector.memset(dbg_nr[:], 0.0)

    # ---------------- filler units ----------------
    def qkv_pass(j, which):
        def unit():
            ps = p["fps"].tile([128, SQ], F32, tag="fps", name=f"p{which}_{j}")
            for kc in range(8):
                if which == 0:
                    w = wqk_sb[:, kc, 0:128]
                elif which == 1:
                    w = wqk_sb[:, kc, 128:256]
                else:
                    w = wv_sb[:, kc, :]
                nc.tensor.matmul(ps[:], w, xts[j][kc][:],
                                 start=(kc == 0), stop=(kc == 7))
            sl = slice(SQ * j, SQ * (j + 1))
            if which == 0:
                nc.scalar.copy(qT[:, sl], ps[:])
            elif which == 1:
                nc.scalar.copy(kT[:, sl], ps[:])
            else:
                vt = p["vtst"].tile([128, SQ], BF16, tag="vt", name=f"vt{j}")
                nc.vector.tensor_copy(vt[:], ps[:])
                vt_stage[j] = vt
        return unit

    def transp_unit(j):
        def unit():
            tp = p["fps"].tile([128, SQ], BF16, tag="fps", name=f"tp{j}")
            vt = vt_stage.pop(j)
            for t4 in range(4):
                csl = slice(128 * t4, 128 * (t4 + 1))
                nc.tensor.transpose(tp[:, csl], vt[:, csl], ident[:])
                jj = 4 * j + t4
                dst = v_all[:, 130 * jj:130 * jj + 130].rearrange(
                    "p (t c) -> p t c", c=65)[:, :, 0:64]
                src = tp[:, csl].rearrange("p (t c) -> p t c", c=64)
                nc.vector.tensor_copy(dst, src)
        return unit

    def rope_unit(j, which):
        # the x1/x2 32-row stripe swap runs on the PE as a permutation
        # matmul (an SBUF->SBUF DMA here loses a race against the DVE
        # consumer: sparse NaNs in the first stripe)
        def unit():
            t = qT if which == 0 else kT
            sl = slice(SQ * j, SQ * (j + 1))
            sw = p["fps"].tile([128, SQ], F32, tag="fps", name=f"sw{which}_{j}")
            nc.tensor.matmul(sw[:], perm_sb[:], t[:, sl], start=True, stop=True)
            ta = p["ropep"].tile([128, SQ], BF16, tag="ta", name=f"ta{which}_{j}")
            tb = p["ropep"].tile([128, SQ], BF16, tag="tb", name=f"tb{which}_{j}")
            nc.vector.tensor_tensor(ta[:], t[:, sl], cos_sb[:, sl], op=mul)
            nc.vector.tensor_tensor(tb[:], sw[:], sin_sb[:, sl], op=mul)
            nc.vector.tensor_tensor(t[:, sl], ta[:], tb[:], op=addop)
        return unit

    def norm_unit(b, h, out_ps, rec):
        def unit():
            qsl = slice(SQ * b, SQ * (b + 1))
            bc = p["fps"].tile([64, SQ], F32, tag="fps", name=f"bc{b}_{h}")
            for hf in (slice(0, SQ // 2), slice(SQ // 2, SQ)):
                nc.tensor.matmul(bc[:, hf], ones_sb[64:65, :], rec[64:65, hf],
                                 start=True, stop=True)
            nrm = p["nrmp"].tile([64, SQ], BF16, tag="nrm", name=f"nr{b}_{h}")
            nc.scalar.copy(nrm[:], out_ps[0:64, :])
            if dbg_nr is not None:
                dsl = slice(SQ * (2 * b + h), SQ * (2 * b + h + 1))
                nc.vector.tensor_copy(dbg_nr[0:64, dsl], nrm[:])
                nc.vector.tensor_copy(dbg_nr[64:65, dsl], rec[64:65, :])
            t64 = p["t64p"].tile([64, SQ], BF16, tag="t64", name=f"t6{b}_{h}")
            nc.vector.tensor_tensor(t64[:], nrm[:], bc[:], op=mul)
            nc.sync.dma_start(outT[64 * h:64 * h + 64, qsl], t64[:])
        return unit

    def oproj_unit(b, m):
        def unit():
            msl = slice(SQ * b + 128 * m, SQ * b + 128 * (m + 1))
            for nh2 in range(2):
                nsl = slice(512 * nh2, 512 * (nh2 + 1))
                y_ps = p["fps"].tile([128, SQ], F32, tag="fps",
                                     name=f"y{b}_{m}_{nh2}")
                nc.tensor.matmul(y_ps[:], outT[:, msl], wo_sb[:, nsl],
                                 start=True, stop=True)
                y_sb = p["ysbp"].tile([128, SQ], BF16, tag="ysb",
                                      name=f"ys{b}_{m}_{nh2}")
                nc.vector.tensor_copy(y_sb[:], y_ps[:])
                (nc.sync if nh2 == 0 else nc.gpsimd).dma_start(
                    y.ap()[msl, nsl], y_sb[:])
        return unit

    # ---------------- prologue: project blocks 0 and 1 inline ----------------
    for j in (0, 1):
        for which in range(3):
            qkv_pass(j, which)()
        transp_unit(j)()
        rope_unit(j, 0)()
        rope_unit(j, 1)()

    # ---------------- fused attention + filler main loop ----------------
    fq = deque()       # high-priority filler (projections, norms)
    fq2 = deque()      # output-projection backlog

    def emit_units(b):
        n = 0
        while fq and n < 3:
            fq.popleft()()
            n += 1
        reserve = OPROJ_RESERVE if b < OPROJ_FREE_B else 0
        while fq2 and len(fq2) > reserve and n < 3:
            fq2.popleft()()
            n += 1

    for b in range(NB512):
        j = b + 2
        if j < NB512:
            fq.append(qkv_pass(j, 0))
            fq.append(qkv_pass(j, 1))
            fq.append(qkv_pass(j, 2))
            fq.append(transp_unit(j))
            fq.append(rope_unit(j, 0))
            fq.append(rope_unit(j, 1))
        if b + 3 < NB512:
            load_x(b + 3, (nc.sync, nc.gpsimd))

        nk = 4 * b + 4
        qsl = slice(SQ * b, SQ * (b + 1))
        for h in range(2):
            rh = slice(64 * h, 64 * h + 64)
            out_ps = p["smps"].tile([65, SQ], F32, tag="sm", name=f"o{b}_{h}")

            def emit_pv(pt, lo, g0):
                for j2 in range(GK):
                    k = g0 + j2
                    nc.tensor.matmul(out_ps[:, lo[j2]:SQ],
                                     v_all[:, 130 * k + 65 * h:
                                           130 * k + 65 * h + 65],
                                     pt[:, SQ * j2 + lo[j2]:SQ * (j2 + 1)],
                                     start=(k == 0), stop=(k == nk - 1))

            prev = None
            for g0 in range(0, nk, GK):
                sc = p["scps"].tile([128, GK * SQ], F32, tag="sc",
                                    name=f"sc{b}_{h}_{g0}")
                lo = {}
                for j2 in range(GK):
                    k = g0 + j2
                    lo[j2] = 128 * (k - 4 * b) if k >= 4 * b else 0
                    # write-lo: j2=1 always writes its full 512 columns so
                    # the single per-group activation reads a contiguous,
                    # fully-written [lo0, 1024) range (PV/tri still use the
                    # real lo; the extra columns are never streamed).
                    wlo = lo[j2] if j2 == 0 else 0
                    ksl = slice(SK * k, SK * (k + 1))
                    osl = slice(SQ * j2 + wlo, SQ * (j2 + 1))
                    nc.tensor.matmul(sc[:, osl], kT[rh, ksl],
                                     qT[rh, slice(SQ * b + wlo,
                                                  SQ * (b + 1))],
                                     start=True, stop=True)
                pt = p["ptp"].tile([128, GK * SQ], BF16, tag="pt",
                                   name=f"pt{b}_{h}_{g0}")
                nc.scalar.activation(pt[:, lo[0]:SQ * GK], sc[:, lo[0]:SQ * GK],
                                     exp, scale=0.125)
                for j2 in range(GK):
                    k = g0 + j2
                    if k >= 4 * b:
                        dsl = slice(SQ * j2 + lo[j2], SQ * j2 + lo[j2] + 128)
                        nc.vector.tensor_tensor(pt[:, dsl], pt[:, dsl],
                                                tri_sb[:], op=mul)
                emit_units(b)
                if prev is not None:
                    emit_pv(*prev)
                prev = (pt, dict(lo), g0)
            emit_pv(*prev)

            rec_f = p["recp"].tile([65, SQ], F32, tag="rec_f", name=f"rf{b}{h}")
            rec = p["recp"].tile([65, SQ], BF16, tag="rec", name=f"rc{b}{h}")
            nc.vector.reciprocal(rec_f[64:65, :], out_ps[64:65, :])
            nc.vector.tensor_copy(rec[64:65, :], rec_f[64:65, :])
            fq.appendleft(norm_unit(b, h, out_ps, rec))
        for m in range(4):
            fq2.append(oproj_unit(b, m))

    # ---------------- epilogue ----------------
    while fq:
        fq.popleft()()
    while fq2:
        fq2.popleft()()

    if dbg is not None:
        # qT/kT slots repurposed to carry the norm-unit dump (nrm rows 0:64,
        # rec row 64) for bh 0-7 and 8-15 respectively
        nc.sync.dma_start(dbg["qT"].ap(), dbg_nr[:, 0:S])
        nc.sync.dma_start(dbg["kT"].ap(), dbg_nr[:, S:2 * S])
        nc.sync.dma_start(dbg["v_all"].ap(), v_all[:])
        nc.sync.dma_start(dbg["outT"].ap(), outT[:])

    for pl in reversed(ctx_pools):
        pl.__exit__(None, None, None)


_CACHED = None


def _build():
    global _CACHED
    if _CACHED is not None:
        return _CACHED
    nc = bacc.Bacc("TRN2", target_bir_lowering=False, debug=False)
    xT = nc.dram_tensor("xT", [D, S], BF16, kind="ExternalInput")
    wqkT = nc.dram_tensor("wqkT", [D, 256], BF16, kind="ExternalInput")
    wvT = nc.dram_tensor("wvT", [D, 128], BF16, kind="ExternalInput")
    woT = nc.dram_tensor("woT", [128, D], BF16, kind="ExternalInput")
    cosT = nc.dram_tensor("cosT", [128, S], BF16, kind="ExternalInput")
    sinT = nc.dram_tensor("sinT", [128, S], BF16, kind="ExternalInput")
    tri = nc.dram_tensor("tri", [128, 128], BF16, kind="ExternalInput")
    perm = nc.dram_tensor("perm", [128, 128], BF16, kind="ExternalInput")
    y = nc.dram_tensor("y", [S, D], BF16, kind="ExternalOutput")
    dbg = None
    if os.environ.get("KERN_DEBUG"):
        dbg = {
            "qT": nc.dram_tensor("dbg_qT", [128, S], BF16, kind="ExternalOutput"),
            "kT": nc.dram_tensor("dbg_kT", [128, S], BF16, kind="ExternalOutput"),
            "v_all": nc.dram_tensor("dbg_v_all", [128, NB128 * 130], BF16,
                                    kind="ExternalOutput"),
            "outT": nc.dram_tensor("dbg_outT", [128, S], BF16,
                                   kind="ExternalOutput"),
        }
    with tile.TileContext(nc) as tc:
        _emit(tc, xT, wqkT, wvT, woT, cosT, sinT, tri, perm, y, dbg=dbg)
    nc.compile()
    _CACHED = nc
    return nc


def _host_prep(x, token_positions, Wq, Wk, Wv, Wo):
    x = np.asarray(x, dtype=np.float32).reshape(S, D)
    xT = np.ascontiguousarray(x.T).astype(NPBF16)

    pos = np.asarray(token_positions).reshape(S).astype(np.float32)
    inv = (np.float32(10000.0) **
           (-np.arange(0, DK // 2, dtype=np.float32) * np.float32(2.0 / DK)))
    ang = pos[None, :] * inv[:, None]          # [32, S]
    cosF = np.cos(ang).astype(np.float32)
    sinF = np.sin(ang).astype(np.float32)
    cosT = np.ascontiguousarray(np.tile(cosF, (4, 1))).astype(NPBF16)
    sinT = np.ascontiguousarray(np.tile(
        np.concatenate([-sinF, sinF], axis=0), (2, 1))).astype(NPBF16)  # signed

    ii = np.arange(128)[:, None]
    uu = np.arange(128)[None, :]
    tri = (uu >= ii).astype(NPBF16)             # [128, 128] triangle

    # stripe-swap permutation for RoPE: out[m] = in[m with 32-row block
    # pairs swapped]; used as matmul stationary (perm[k, m] = 1 iff k = swap(m))
    swap = (np.arange(128) // 32 ^ 1) * 32 + np.arange(128) % 32
    perm = np.zeros((128, 128), dtype=NPBF16)
    perm[swap, np.arange(128)] = 1.0

    Wq = np.asarray(Wq, dtype=np.float32)
    Wk = np.asarray(Wk, dtype=np.float32)
    Wv = np.asarray(Wv, dtype=np.float32)
    Wo = np.asarray(Wo, dtype=np.float32)

    in_maps = []
    for c in range(N_CORES):
        idx = []
        for hl in range(2):   # per head: 32 even channels then 32 odd channels
            idx += [64 * (2 * c + hl) + 2 * j for j in range(32)]
            idx += [64 * (2 * c + hl) + 2 * j + 1 for j in range(32)]
        wq_c = Wq[idx, :]                       # [128, 1024]
        wk_c = Wk[idx, :]
        wqkT = np.ascontiguousarray(
            np.concatenate([wq_c.T, wk_c.T], axis=1)).astype(NPBF16)  # [1024, 256]
        wvT = np.ascontiguousarray(
            Wv[128 * c:128 * (c + 1), :].T).astype(NPBF16)  # [1024, 128]
        woT = np.ascontiguousarray(
            Wo[:, 128 * c:128 * (c + 1)].T).astype(NPBF16)  # [128, 1024]
        in_maps.append({
            "xT": xT, "wqkT": wqkT, "wvT": wvT, "woT": woT,
            "cosT": cosT, "sinT": sinT, "tri": tri, "perm": perm,
        })
    return in_maps


def run(x, token_positions, Wq, Wk, Wv, Wo, trace=False):
    nc = _build()
    in_maps = _host_prep(x, token_positions, Wq, Wk, Wv, Wo)
    res = run_bass_kernel_spmd(nc, in_maps, core_ids=list(range(N_CORES)),
                               trace=trace)
    y = np.zeros((S, D), dtype=np.float32)
    for c in range(N_CORES):
        y += np.asarray(res.results[c]["y"], dtype=np.float32)
    return y.reshape(1, S, D), res


def kernel(x, token_positions, Wq, Wk, Wv, Wo):
    y, _ = run(x, token_positions, Wq, Wk, Wv, Wo)
    return y
